# revision 10
# baseline (speedup 1.0000x reference)
"""RealFormer-style MultiHeadAttention on 8 Trainium2 NeuronCores.

Reference computation (B=8, S=1024, D=1024, H=16, HD=64):
    q = split_heads(hidden @ Wq + bq); k = ...; v = ...
    scores = (q @ k^T) * HD**-0.5 + attn_mask + prev_attn_weights
    out    = merge_heads(softmax(scores) @ v)

Sharding: pure data-parallel over batch — one batch element per core,
no collectives.

Per-core kernel design (all matmul operands fp16, accumulation fp32):
  * Host folds SCALE into Wq, attn_mask into prev, pre-transposes hidden
    and casts the streamed operands to fp16.
  * qT,kT ([D,S], head-dim on partitions) and v ([S,D]) computed on PE.
    v is stored interleaved as vx[S, H*65] where column 65h+64 is 1.0 so
    the PV matmul also produces softmax row-sums for free.
  * Per head: PE transposes prev[q,k] tiles into PSUM (start=True), then
    scoresT[k,q] = kT^T @ qT accumulates on top (start=False) — the
    additive-prev costs no separate vector pass.
  * probsT = exp(scoresT - 10) on ScalarE straight out of PSUM into fp16
    SBUF.  The constant shift keeps exp() in fp16 range and cancels in
    the normalization, so no row-max pass is needed.
  * ctxT[65, q] = vx^T @ probsT accumulated over k; tiny PE re-transpose
    to [q, 65]; VectorE reciprocal of column 64 + per-partition scale
    writes the final fp32 output.
"""

import sys

if "/opt/trn_rl_repo" not in sys.path:
    sys.path.insert(0, "/opt/trn_rl_repo")

import numpy as np

B, S, D, H = 8, 1024, 1024, 16
HD = D // H
SCALE = HD**-0.5
P = 128
N_CORES = 8
EXP_SHIFT = 10.0

_compiled = {}


def _build(use_bias: bool):
    import concourse.bacc as bacc
    import concourse.mybir as mybir
    import concourse.tile as tile
    from concourse.masks import make_identity

    f16 = mybir.dt.float16
    f32 = mybir.dt.float32
    Exp = mybir.ActivationFunctionType.Exp

    nc = bacc.Bacc("TRN2", target_bir_lowering=False, debug=False)

    hT_d = nc.dram_tensor("hiddenT", (D, S), f16, kind="ExternalInput").ap()
    w_d = {
        name: nc.dram_tensor(name, (D, D), f16, kind="ExternalInput").ap()
        for name in ("wq", "wk", "wv")
    }
    prev_d = nc.dram_tensor("prevm", (H, S, S), f16, kind="ExternalInput").ap()
    b_d = {}
    if use_bias:
        b_d = {
            name: nc.dram_tensor(name, (1, D), f16, kind="ExternalInput").ap()
            for name in ("bq", "bk", "bv")
        }
    out_d = nc.dram_tensor("out", (S, D), f32, kind="ExternalOutput").ap()

    with tile.TileContext(nc) as tc:
        with (
            tc.tile_pool(name="big", bufs=1) as big,
            tc.tile_pool(name="wpool", bufs=10) as wpool,
            tc.tile_pool(name="ppool", bufs=8) as ppool,
            tc.tile_pool(name="probs", bufs=2) as probs_pool,
            tc.tile_pool(name="small", bufs=3) as small,
            tc.tile_pool(name="const", bufs=1) as const_pool,
        ):
            ident = const_pool.tile([P, P], f16)
            make_identity(nc, ident)
            neg_shift = const_pool.tile([P, 1], f32)
            nc.any.memset(neg_shift, -EXP_SHIFT)
            if use_bias:
                ones_row = const_pool.tile([1, 512], f16)
                nc.any.memset(ones_row, 1.0)
                b_sb = {}
                for name in ("bq", "bk", "bv"):
                    bt = const_pool.tile([1, D], f16, name=f"bsb_{name}")
                    nc.sync.dma_start(bt, b_d[name])
                    b_sb[name] = bt

            hidT = big.tile([P, 8, S], f16, tag="hidT")
            nc.sync.dma_start(hidT, hT_d.rearrange("(do di) s -> di do s", di=P))

            qT = big.tile([P, 8, S], f16, tag="qT")
            kT = big.tile([P, 8, S], f16, tag="kT")
            vx = big.tile([P, 8, H * 65], f16, tag="vx")
            out_sb = big.tile([P, 8, D], f32, tag="osb")

            # ---- projections (scoped f32 PSUM pool, released before heads) ----
            vx_view = vx.rearrange("p t (h c) -> p t h c", c=65)
            nc.any.memset(vx_view[:, :, :, 64], 1.0)
            with tc.tile_pool(name="ps_proj", bufs=2, space="PSUM") as ps_proj:
                # q/k: dest[dout, s] = W^T @ hidden^T
                for pname, dest in (("q", qT), ("k", kT)):
                    wts = []
                    for kt in range(8):
                        wt = wpool.tile([P, D], f16, tag="w", name=f"w_{pname}{kt}")
                        nc.sync.dma_start(
                            wt, w_d["w" + pname][kt * P : (kt + 1) * P, :]
                        )
                        wts.append(wt)
                    for po in range(8):
                        pt = ps_proj.tile([P, S], f32, tag="psb", name=f"ps_{pname}{po}")
                        for half in range(2):
                            hs = slice(half * 512, half * 512 + 512)
                            for kt in range(8):
                                nc.tensor.matmul(
                                    pt[:, hs],
                                    lhsT=wts[kt][:, po * P : (po + 1) * P],
                                    rhs=hidT[:, kt, hs],
                                    start=(kt == 0),
                                    stop=(kt == 7 and not use_bias),
                                )
                            if use_bias:
                                nc.tensor.matmul(
                                    pt[:, hs],
                                    lhsT=b_sb["b" + pname][:, po * P : (po + 1) * P],
                                    rhs=ones_row,
                                    start=False,
                                    stop=True,
                                )
                        nc.vector.tensor_copy(dest[:, po, :], pt[:])

                # v: v[s, dout] interleaved into vx with the ones column
                wts = []
                for kt in range(8):
                    wt = wpool.tile([P, D], f16, tag="w", name=f"w_v{kt}")
                    nc.sync.dma_start(wt, w_d["wv"][kt * P : (kt + 1) * P, :])
                    wts.append(wt)
                for pt_i in range(8):
                    pv = ps_proj.tile([P, D], f32, tag="psb", name=f"ps_v{pt_i}")
                    for half in range(2):
                        hs = slice(half * 512, half * 512 + 512)
                        for dt in range(8):
                            nc.tensor.matmul(
                                pv[:, hs],
                                lhsT=hidT[:, dt, pt_i * P : (pt_i + 1) * P],
                                rhs=wts[dt][:, hs],
                                start=(dt == 0),
                                stop=(dt == 7 and not use_bias),
                            )
                        if use_bias:
                            nc.tensor.matmul(
                                pv[:, hs],
                                lhsT=ones_row[:, :P],
                                rhs=b_sb["bv"][:, hs],
                                start=False,
                                stop=True,
                            )
                    nc.vector.tensor_copy(
                        vx_view[:, pt_i, :, 0:64],
                        pv.rearrange("p (h e) -> p h e", e=64),
                    )

            # ---- per-head attention ----
            with (
                tc.tile_pool(name="ps_sc", bufs=2, space="PSUM") as ps_sc,
                tc.tile_pool(name="ps_ctx", bufs=1, space="PSUM") as ps_ctx,
                tc.tile_pool(name="ps_t", bufs=2, space="PSUM") as ps_t,
            ):
                for h in range(16):
                    r, t = h % 2, h // 2
                    rs = slice(r * 64, (r + 1) * 64)
                    # prevm is shipped pre-transposed by the host: [h, k, q]
                    pv_ap = prev_d[h].rearrange("(ko ki) q -> ki ko q", ki=P)
                    prev_sb = []
                    for j in range(4):
                        pj = ppool.tile(
                            [P, 2, S], f16, tag="prev", name=f"prev_{h}_{j}"
                        )
                        nc.sync.dma_start(pj, pv_ap[:, j * 2 : (j + 1) * 2, :])
                        prev_sb.append(pj)

                    probsT = probs_pool.tile(
                        [P, 8, S], f16, tag="probsT", name=f"probsT_{h}"
                    )
                    for kt in range(8):
                        ks = slice(kt * P, (kt + 1) * P)
                        ps = ps_sc.tile([P, S], f32, tag="pssc", name=f"ps_s_{h}_{kt}")
                        for half in range(2):
                            hs = slice(half * 512, half * 512 + 512)
                            # inject prev^T: identity (stationary) @ prevT chunk
                            nc.tensor.matmul(
                                ps[:, hs],
                                lhsT=ident,
                                rhs=prev_sb[kt // 2][:, kt % 2, hs],
                                start=True,
                                stop=False,
                                skip_group_check=True,
                            )
                            # scoresT accumulate on top
                            nc.tensor.matmul(
                                ps[:, hs],
                                lhsT=kT[rs, t, ks],
                                rhs=qT[rs, t, hs],
                                start=False,
                                stop=True,
                                skip_group_check=True,
                            )
                        nc.scalar.activation(
                            probsT[:, kt, :], ps[:], Exp, bias=neg_shift
                        )

                    pc = ps_ctx.tile([65, S], f32, tag="psc", name=f"ps_c_{h}")
                    for half in range(2):
                        hs = slice(half * 512, half * 512 + 512)
                        for kt in range(8):
                            nc.tensor.matmul(
                                pc[:, hs],
                                lhsT=vx[:, kt, h * 65 : (h + 1) * 65],
                                rhs=probsT[:, kt, hs],
                                start=(kt == 0),
                                stop=(kt == 7),
                            )
                    ctxT_sb = small.tile([65, S], f16, tag="ctxT", name=f"ctxT_{h}")
                    nc.vector.tensor_copy(ctxT_sb, pc)
                    for qt in range(8):
                        ptt = ps_t.tile([P, 65], f16, tag="pst", name=f"ps_t_{h}_{qt}")
                        nc.tensor.matmul(
                            ptt,
                            lhsT=ctxT_sb[:, qt * P : (qt + 1) * P],
                            rhs=ident[0:65, 0:65],
                            is_transpose=True,
                        )
                        rc = small.tile([P, 1], f32, tag="recip", name=f"rc_{h}_{qt}")
                        nc.vector.reciprocal(rc, ptt[:, 64:65])
                        nc.vector.tensor_scalar_mul(
                            out_sb[:, qt, h * 64 : (h + 1) * 64], ptt[:, 0:64], rc
                        )

            nc.sync.dma_start(out_d.rearrange("(qo qi) d -> qi qo d", qi=P), out_sb)

    nc.compile()
    return nc


def _get_compiled(use_bias: bool):
    if use_bias not in _compiled:
        _compiled[use_bias] = _build(use_bias)
    return _compiled[use_bias]


def _prepare_in_maps(
    hidden_states, attn_mask, prev_attn_weights, Wq, bq, Wk, bk, Wv, bv, use_bias
):
    hs = np.asarray(hidden_states, np.float32)
    mask = np.asarray(attn_mask, np.float32)
    prev = np.asarray(prev_attn_weights, np.float32)

    wq16 = (np.asarray(Wq, np.float32) * SCALE).astype(np.float16)
    wk16 = np.asarray(Wk, np.float32).astype(np.float16)
    wv16 = np.asarray(Wv, np.float32).astype(np.float16)

    # fold mask in, pre-transpose to [b, h, k, q], cast to fp16
    if np.any(mask):
        prevm = (prev + mask).transpose(0, 1, 3, 2).astype(np.float16)
    else:
        prevm = prev.transpose(0, 1, 3, 2).astype(np.float16)
    hT = np.ascontiguousarray(hs.transpose(0, 2, 1)).astype(np.float16)

    in_maps = []
    for b in range(N_CORES):
        m = {
            "hiddenT": np.ascontiguousarray(hT[b]),
            "wq": wq16,
            "wk": wk16,
            "wv": wv16,
            "prevm": np.ascontiguousarray(prevm[b]),
        }
        if use_bias:
            m["bq"] = (np.asarray(bq, np.float32) * SCALE).astype(np.float16)[None, :]
            m["bk"] = np.asarray(bk, np.float32).astype(np.float16)[None, :]
            m["bv"] = np.asarray(bv, np.float32).astype(np.float16)[None, :]
        in_maps.append(m)
    return in_maps


def kernel(hidden_states, attn_mask, prev_attn_weights, Wq, bq, Wk, bk, Wv, bv):
    from concourse.bass_utils import run_bass_kernel_spmd

    use_bias = bool(np.any(bq) or np.any(bk) or np.any(bv))
    nc = _get_compiled(use_bias)
    in_maps = _prepare_in_maps(
        hidden_states, attn_mask, prev_attn_weights, Wq, bq, Wk, bk, Wv, bv, use_bias
    )
    res = run_bass_kernel_spmd(nc, in_maps, core_ids=list(range(N_CORES)))
    return np.stack([res.results[b]["out"] for b in range(N_CORES)]).astype(np.float32)
